# revision 1
# baseline (speedup 1.0000x reference)
"""GPT-Neo self-attention on 8 NeuronCores (Trainium2, Bass/Tile).

Sharding: tensor-parallel over (batch, head-group). Core i handles batch
i//4 and head-group i%4 (3 of 12 heads). Each core computes a partial
out-projection [S, D]; the host sums the 4 partials per batch.

Per-core math (B=2, S=2048, D=768, H=12, HD=64, 3 heads/core):
  qT,kT = W @ x.T                  # heads 0,1 stacked on partitions; head 2 solo
  v     = x @ WvT (+ ones column)  # natural [S, 65] per head
  sT    = kT-stationary x qT-moving -> scoresT [sk, sq] (causal blocks only)
  pT    = exp(sT + diag_mask + pad_bias)
  outT  = [v|1].T @ pT             # [65, sq]; row 64 = softmax denominators
  onorm = outT * (1/outT[64]) bcast
  y    += onorm.T @ [WoT_h; bo]    # bo folded as 65th contraction row (h==0)

Matmul operands are bf16 (full PE rate); accumulation/softmax in fp32.
"""

import numpy as np
import ml_dtypes
from contextlib import ExitStack

import concourse.bass as bass
from concourse import bacc
import concourse.mybir as mybir
import concourse.tile as tile
from concourse.bass_utils import run_bass_kernel_spmd

B, S, D, H = 2, 2048, 768, 12
HD = 64          # head dim
HPC = 3          # heads per core
NCORES = 8
NEG = -1.0e30
F32 = mybir.dt.float32
BF16 = mybir.dt.bfloat16

KT = D // 128    # 6 k-tiles over the model dim
SQT = S // 128   # 16 seq tiles of 128
CH = S // 512    # 4 seq chunks of 512


def build_nc():
    nc = bacc.Bacc(None, target_bir_lowering=False)

    xT = nc.declare_dram_parameter("xT", [D, S], BF16, isOutput=False)
    # cols 0:128 = [q0|q1] pair, 128:256 = [k0|k1] pair, 256:320 = q2, 320:384 = k2
    wqk = nc.declare_dram_parameter("wqk", [D, HPC * 128], BF16, isOutput=False)
    # WvT slice for the 3 heads, padded to 256 cols  ([768, 256])
    wv = nc.declare_dram_parameter("wv", [D, 256], BF16, isOutput=False)
    # per head: 65 rows x 768 (row 64 = bo for h==0 on group-0 cores, else 0)
    wo = nc.declare_dram_parameter("wo", [HPC * 65, D], BF16, isOutput=False)
    # 4 causal diag-mask tiles stacked: [512, 512]
    masks = nc.declare_dram_parameter("masks", [4 * 128, 512], F32, isOutput=False)
    # padding bias per key position, packed [128, 16] (col j = positions 128j..)
    pbias = nc.declare_dram_parameter("pbias", [128, SQT], F32, isOutput=False)
    y = nc.declare_dram_parameter("y", [S, D], F32, isOutput=True)

    with tile.TileContext(nc) as tc:
        with ExitStack() as ctx:
            persist = ctx.enter_context(tc.tile_pool(name="persist", bufs=1))
            work = ctx.enter_context(tc.tile_pool(name="work", bufs=4))
            outp = ctx.enter_context(tc.tile_pool(name="outp", bufs=2))
            psum_s = ctx.enter_context(
                tc.tile_pool(name="psum_s", bufs=3, space="PSUM"))
            psum_o = ctx.enter_context(
                tc.tile_pool(name="psum_o", bufs=2, space="PSUM"))
            psum_p = ctx.enter_context(
                tc.tile_pool(name="psum_p", bufs=1, space="PSUM"))

            # ---- persistent SBUF tiles ----
            xT_sb = [persist.tile([128, S], BF16, tag=f"xT{k}", name=f"xT{k}")
                     for k in range(KT)]
            wqk_sb = [persist.tile([128, HPC * 128], BF16, tag=f"wqk{k}",
                                   name=f"wqk{k}") for k in range(KT)]
            wv_sb = [persist.tile([128, 256], BF16, tag=f"wv{k}", name=f"wv{k}")
                     for k in range(KT)]
            wo_sb = [persist.tile([65, D], BF16, tag=f"wo{h}", name=f"wo{h}")
                     for h in range(HPC)]
            mask_sb = [persist.tile([128, 512], F32, tag=f"mask{m}",
                                    name=f"mask{m}") for m in range(4)]
            pb_sb = persist.tile([128, SQT], F32, tag="pb", name="pb")
            # heads 0,1 stacked on partitions (0:64 / 64:128); head 2 alone
            q01_sb = persist.tile([128, S], BF16, tag="q01", name="q01")
            k01_sb = persist.tile([128, S], BF16, tag="k01", name="k01")
            q2_sb = persist.tile([64, S], BF16, tag="q2", name="q2")
            k2_sb = persist.tile([64, S], BF16, tag="k2", name="k2")
            # v for 3 heads: per head 16 sk-tiles x 65 cols (col 64 == 1.0)
            v_sb = persist.tile([128, HPC * SQT * 65], BF16, tag="v", name="v")
            onorm_sb = [persist.tile([65, S], BF16, tag=f"onorm{h}",
                                     name=f"onorm{h}") for h in range(HPC)]

            # ---- small-weight loads ----
            for k in range(KT):
                nc.sync.dma_start(out=wqk_sb[k][:], in_=wqk[128 * k:128 * (k + 1), :])
                nc.sync.dma_start(out=wv_sb[k][:], in_=wv[128 * k:128 * (k + 1), :])
            for h in range(HPC):
                nc.sync.dma_start(out=wo_sb[h][:], in_=wo[65 * h:65 * (h + 1), :])
            for m in range(4):
                nc.sync.dma_start(out=mask_sb[m][:],
                                  in_=masks[128 * m:128 * (m + 1), :])
            nc.sync.dma_start(out=pb_sb[:], in_=pbias[:, :])
            nc.vector.memset(v_sb[:], 1.0)

            def v_ap(h, j, width=65):
                off = (h * SQT + j) * 65
                return v_sb[:, off:off + width]

            for c in range(CH):
                cs = slice(512 * c, 512 * (c + 1))
                # ---- stream in this column-chunk of x.T ----
                for k in range(KT):
                    nc.sync.dma_start(out=xT_sb[k][:, cs],
                                      in_=xT[128 * k:128 * (k + 1), cs])

                # ---- q/k projections for this chunk ----
                proj_plan = [(0, 128, q01_sb), (128, 128, k01_sb),
                             (256, 64, q2_sb), (320, 64, k2_sb)]
                for off, width, dst in proj_plan:
                    ps = psum_s.tile([128, 512], F32, tag="ps", name="ps")
                    for k in range(KT):
                        nc.tensor.matmul(
                            out=ps[0:width, :],
                            lhsT=wqk_sb[k][:, off:off + width],
                            rhs=xT_sb[k][:, cs],
                            start=(k == 0), stop=(k == KT - 1))
                    nc.vector.tensor_copy(out=dst[:, cs], in_=ps[0:width, :])

                # ---- v projection for the 4 sk-tiles in this chunk ----
                for j in range(4 * c, 4 * c + 4):
                    pv = psum_s.tile([128, 256], F32, tag="ps", name="psv")
                    for k in range(KT):
                        nc.tensor.matmul(
                            out=pv[:],
                            lhsT=xT_sb[k][:, 128 * j:128 * (j + 1)],
                            rhs=wv_sb[k][:],
                            start=(k == 0), stop=(k == KT - 1))
                    for h in range(HPC):
                        nc.vector.tensor_copy(out=v_ap(h, j, 64),
                                              in_=pv[:, 64 * h:64 * (h + 1)])

                # ---- attention for chunk c ----
                # heads 0,1 interleaved: their K=64 score matmuls sit on PE
                # row-groups 0:64 / 64:128 and can run concurrently
                qk_views = [
                    (k01_sb[0:64, :], q01_sb[0:64, :]),
                    (k01_sb[64:128, :], q01_sb[64:128, :]),
                    (k2_sb[:, :], q2_sb[:, :]),
                ]

                def normalize(h, po):
                    # rows / row64 (row64 becomes exactly 1.0)
                    recip = work.tile([1, 512], F32, tag="recip", name="recip")
                    nc.vector.reciprocal(out=recip[:], in_=po[64:65, :])
                    bcast = work.tile([65, 512], F32, tag="bcast", name="bcast")
                    nc.gpsimd.partition_broadcast(bcast[:], recip[0:1, :])
                    nc.vector.tensor_mul(out=onorm_sb[h][:, cs], in0=po[:],
                                         in1=bcast[:])

                po01 = [psum_o.tile([65, 512], F32, tag="po", name="po")
                        for _ in range(2)]
                for j in range(4 * c + 4):
                    pts = []
                    for h in range(2):
                        k_view, q_view = qk_views[h]
                        ss = psum_s.tile([128, 512], F32, tag="ps", name="ss")
                        nc.tensor.matmul(
                            out=ss[:],
                            lhsT=k_view[:, 128 * j:128 * (j + 1)],
                            rhs=q_view[:, cs],
                            start=True, stop=True)
                        if j // 4 == c:  # diagonal chunk -> causal mask
                            nc.vector.tensor_add(out=ss[:], in0=ss[:],
                                                 in1=mask_sb[j % 4][:])
                        pt = work.tile([128, 512], BF16, tag="pt", name="pt")
                        nc.scalar.activation(
                            out=pt[:], in_=ss[:],
                            func=mybir.ActivationFunctionType.Exp,
                            bias=pb_sb[:, j:j + 1])
                        pts.append(pt)
                    for h in range(2):
                        nc.tensor.matmul(
                            out=po01[h][:], lhsT=v_ap(h, j), rhs=pts[h][:],
                            start=(j == 0), stop=(j == 4 * c + 3))
                for h in range(2):
                    normalize(h, po01[h])

                k_view, q_view = qk_views[2]
                po2 = psum_o.tile([65, 512], F32, tag="po", name="po")
                for j in range(4 * c + 4):
                    ss = psum_s.tile([128, 512], F32, tag="ps", name="ss")
                    nc.tensor.matmul(
                        out=ss[:],
                        lhsT=k_view[:, 128 * j:128 * (j + 1)],
                        rhs=q_view[:, cs],
                        start=True, stop=True)
                    if j // 4 == c:
                        nc.vector.tensor_add(out=ss[:], in0=ss[:],
                                             in1=mask_sb[j % 4][:])
                    pt = work.tile([128, 512], BF16, tag="pt", name="pt")
                    nc.scalar.activation(
                        out=pt[:], in_=ss[:],
                        func=mybir.ActivationFunctionType.Exp,
                        bias=pb_sb[:, j:j + 1])
                    nc.tensor.matmul(
                        out=po2[:], lhsT=v_ap(2, j), rhs=pt[:],
                        start=(j == 0), stop=(j == 4 * c + 3))
                normalize(2, po2)

                # ---- out-projection for the 4 sq-tiles of this chunk ----
                for t in range(4 * c, 4 * c + 4):
                    ts_ = slice(128 * t, 128 * (t + 1))
                    pp = [psum_p.tile([128, 384], F32, tag=f"pp{half}",
                                      name=f"pp{half}") for half in range(2)]
                    for h in range(HPC):
                        for half in range(2):
                            hs = slice(384 * half, 384 * (half + 1))
                            nc.tensor.matmul(
                                out=pp[half][:],
                                lhsT=onorm_sb[h][:, ts_],
                                rhs=wo_sb[h][:, hs],
                                start=(h == 0), stop=(h == HPC - 1))
                    ot = outp.tile([128, D], F32, tag="ot", name="ot")
                    for half in range(2):
                        hs = slice(384 * half, 384 * (half + 1))
                        nc.vector.tensor_copy(out=ot[:, hs], in_=pp[half][:])
                    nc.sync.dma_start(out=y[ts_, :], in_=ot[:])

    nc.compile()
    return nc


def make_inputs(x, attention_mask, Wq, Wk, Wv, Wo, bo):
    """Per-core input maps (host-side sharding)."""
    bf = ml_dtypes.bfloat16
    # shared across cores
    masks = np.zeros((4 * 128, 512), np.float32)
    kk = np.arange(128)[:, None]
    qq = np.arange(512)[None, :]
    for m in range(4):
        masks[128 * m:128 * (m + 1), :] = np.where(
            qq >= 128 * m + kk, 0.0, NEG).astype(np.float32)

    in_maps = []
    for core in range(NCORES):
        b, g = core // 4, core % 4
        h0, h1, h2 = range(HPC * g, HPC * (g + 1))
        xTb = np.ascontiguousarray(x[b].T).astype(bf)
        wqk = np.empty((D, HPC * 128), np.float32)
        wqk[:, 0:64] = Wq[HD * h0:HD * (h0 + 1), :].T      # q01 pair
        wqk[:, 64:128] = Wq[HD * h1:HD * (h1 + 1), :].T
        wqk[:, 128:192] = Wk[HD * h0:HD * (h0 + 1), :].T   # k01 pair
        wqk[:, 192:256] = Wk[HD * h1:HD * (h1 + 1), :].T
        wqk[:, 256:320] = Wq[HD * h2:HD * (h2 + 1), :].T   # q2
        wqk[:, 320:384] = Wk[HD * h2:HD * (h2 + 1), :].T   # k2
        wv_ = np.zeros((D, 256), np.float32)
        wv_[:, :HPC * HD] = Wv[HD * h0:HD * (h2 + 1), :].T
        wo_ = np.zeros((HPC * 65, D), np.float32)
        for i, h in enumerate((h0, h1, h2)):
            wo_[65 * i:65 * i + 64, :] = Wo[:, HD * h:HD * (h + 1)].T
        if g == 0:  # bo must enter the partial-sum exactly once per batch
            wo_[64, :] = bo
        # padding bias per key position (additive, pre-exp)
        pb = ((1.0 - attention_mask[b].astype(np.float32)) * NEG)
        pbias = np.ascontiguousarray(pb.reshape(SQT, 128).T)
        in_maps.append({"xT": xTb, "wqk": wqk.astype(bf), "wv": wv_.astype(bf),
                        "wo": wo_.astype(bf), "masks": masks, "pbias": pbias})
    return in_maps


_NC_CACHE = {}


def _get_nc():
    if "nc" not in _NC_CACHE:
        _NC_CACHE["nc"] = build_nc()
    return _NC_CACHE["nc"]


def kernel(x, attention_mask, Wq, Wk, Wv, Wo, bo, _trace=False, _trace_kwargs=None):
    x = np.asarray(x, np.float32)
    attention_mask = np.asarray(attention_mask, np.float32)
    Wq, Wk, Wv, Wo, bo = (np.asarray(a, np.float32) for a in (Wq, Wk, Wv, Wo, bo))
    nc = _get_nc()
    in_maps = make_inputs(x, attention_mask, Wq, Wk, Wv, Wo, bo)
    res = run_bass_kernel_spmd(nc, in_maps, list(range(NCORES)),
                               trace=_trace, **(_trace_kwargs or {}))
    parts = [np.asarray(res.results[i]["y"]) for i in range(NCORES)]
    out = np.stack([sum(parts[0:4]), sum(parts[4:8])]).astype(np.float32)
    if _trace:
        return out, res
    return out



# revision 7
# speedup vs baseline: 1.3734x; 1.3734x over previous
"""GPT-Neo self-attention on 8 NeuronCores (Trainium2, Bass/Tile).

Sharding: tensor-parallel over (batch, head-group). Core i handles batch
i//4 and head-group i%4 (3 of 12 heads). Each core computes a partial
out-projection [S, D]; the host sums the 4 partials per batch.

Per-core pipeline (B=2, S=2048, D=768, H=12, HD=64, 3 heads/core):
  qT,kT   = W @ x.T       heads 0,1 stacked on partitions 0:64/64:128
  v       = x @ WvT       natural [sk, 65] per head (col 64 == 1.0)
  sT      = k-stationary, q-moving -> scoresT [sk, sq]; all 3 heads of a
            key-block go to one 3-bank PSUM group
  pT      = exp(sT + pad_bias)   ONE fused ACT per key-block (3D AP over
            the 3 heads); diagonal blocks are column-shrunk and
            triangle-masked by a bf16 0/1 multiply AFTER exp
  oT      = [v|1].T @ pT  accumulated per head in PSUM; row 64 = denom
  onorm   = oT * recip(denom) (reciprocal_approx_fast + partition bcast)
  y      += onorm01.T @ Wo01  +  onorm2bo.T @ Wo2   (2 matmuls per tile)

Projections for chunk c+1/c+2 are emitted ahead so the tensor engine has
filler work while ScalarE grinds the exp stream.  Matmul operands bf16
(full PE rate); accumulation/softmax fp32.
"""

import numpy as np
import ml_dtypes
from contextlib import ExitStack

import concourse.bass as bass
from concourse import bacc
import concourse.mybir as mybir
import concourse.tile as tile
from concourse.bass_utils import run_bass_kernel_spmd

B, S, D, H = 2, 2048, 768, 12
HD = 64          # head dim
HPC = 3          # heads per core
NCORES = 8
NEG = -1.0e30
F32 = mybir.dt.float32
BF16 = mybir.dt.bfloat16

KT = D // 128    # 6 k-tiles over the model dim
SQT = S // 128   # 16 seq tiles of 128
CH = S // 512    # 4 seq chunks of 512


def build_nc():
    nc = bacc.Bacc(None, target_bir_lowering=False)

    xT = nc.declare_dram_parameter("xT", [D, S], BF16, isOutput=False)
    # cols 0:64 = q0, 64:128 = q1, 128:192 = k0, 192:256 = k1,
    #      256:320 = q2, 320:384 = k2
    wqk = nc.declare_dram_parameter("wqk", [D, HPC * 128], BF16, isOutput=False)
    wv = nc.declare_dram_parameter("wv", [D, HPC * HD], BF16, isOutput=False)
    # rows 0:64 = Wo cols of h0 (transposed), 64:128 = h1
    wo01 = nc.declare_dram_parameter("wo01", [128, D], BF16, isOutput=False)
    # rows 0:64 = h2, row 64 = bo (group-0 cores only)
    wo2 = nc.declare_dram_parameter("wo2", [65, D], BF16, isOutput=False)
    # causal triangle: maskt[k, u] = 1.0 if u >= k else 0.0
    maskt = nc.declare_dram_parameter("maskt", [128, 128], BF16, isOutput=False)
    # padding bias per key position, packed [128, 16] (col j = keys 128j..)
    pbias = nc.declare_dram_parameter("pbias", [128, SQT], F32, isOutput=False)
    y = nc.declare_dram_parameter("y", [S, D], F32, isOutput=True)

    with tile.TileContext(nc) as tc:
        with ExitStack() as ctx:
            persist = ctx.enter_context(tc.tile_pool(name="persist", bufs=1))
            work = ctx.enter_context(tc.tile_pool(name="work", bufs=2))
            psum = ctx.enter_context(
                tc.tile_pool(name="psum", bufs=1, space="PSUM"))

            # ---- persistent SBUF tiles ----
            xT_sb = [persist.tile([128, S], BF16, tag=f"xT{k}", name=f"xT{k}")
                     for k in range(KT)]
            wqk_sb = [persist.tile([128, HPC * 128], BF16, tag=f"wqk{k}",
                                   name=f"wqk{k}") for k in range(KT)]
            wv_sb = [persist.tile([128, HPC * HD], BF16, tag=f"wv{k}",
                                  name=f"wv{k}") for k in range(KT)]
            wo01_sb = persist.tile([128, D], BF16, tag="wo01", name="wo01")
            wo2_sb = persist.tile([65, D], BF16, tag="wo2", name="wo2")
            mask_sb = persist.tile([128, 128], BF16, tag="maskt", name="maskt")
            pb_sb = persist.tile([128, SQT], F32, tag="pb", name="pb")
            # heads 0,1 stacked on partitions (0:64 / 64:128); head 2 alone
            q01_sb = persist.tile([128, S], BF16, tag="q01", name="q01")
            k01_sb = persist.tile([128, S], BF16, tag="k01", name="k01")
            q2_sb = persist.tile([64, S], BF16, tag="q2", name="q2")
            k2_sb = persist.tile([64, S], BF16, tag="k2", name="k2")
            # v: [sk 128, j, h, 65]; col 64 of each (j,h) group == 1.0
            v4 = persist.tile([128, SQT, HPC, 65], BF16, tag="v4", name="v4")
            # exp(scores) buffered for the whole chunk: [sk, j, h, sq-col]
            pt4 = persist.tile([128, SQT, HPC, 512], BF16, tag="pt4",
                               name="pt4")
            onorm01 = persist.tile([128, S], BF16, tag="on01", name="on01")
            onorm2 = persist.tile([65, S], BF16, tag="on2", name="on2")
            dn_sb = persist.tile([65, S], F32, tag="dn", name="dn")
            rc_sb = persist.tile([65, S], F32, tag="rc", name="rc")

            # ---- input DMAs, all hoisted ----
            for c in range(CH):
                cs = slice(512 * c, 512 * (c + 1))
                for k in range(KT):
                    nc.sync.dma_start(out=xT_sb[k][:, cs],
                                      in_=xT[128 * k:128 * (k + 1), cs])
            for k in range(KT):
                nc.sync.dma_start(out=wqk_sb[k][:],
                                  in_=wqk[128 * k:128 * (k + 1), :])
                nc.sync.dma_start(out=wv_sb[k][:],
                                  in_=wv[128 * k:128 * (k + 1), :])
            nc.sync.dma_start(out=wo01_sb[:], in_=wo01[:, :])
            nc.sync.dma_start(out=wo2_sb[:], in_=wo2[:, :])
            nc.sync.dma_start(out=mask_sb[:], in_=maskt[:, :])
            nc.sync.dma_start(out=pb_sb[:], in_=pbias[:, :])
            nc.vector.memset(v4[:], 1.0)          # ones col survives v copies
            nc.vector.memset(onorm2[64:65, :], 1.0)   # bo contraction row
            nc.vector.memset(dn_sb[:], 1.0)       # recip-safe filler lanes

            def emit_proj(c):
                """q/k/v projections for chunk c."""
                cs = slice(512 * c, 512 * (c + 1))
                for off, dst in ((0, "q01"), (128, "k01"), (256, "q2k2")):
                    ps = psum.tile([128, 512], F32, tag="op", bufs=2,
                                   name="psqk")
                    for k in range(KT):
                        nc.tensor.matmul(
                            out=ps[:],
                            lhsT=wqk_sb[k][:, off:off + 128],
                            rhs=xT_sb[k][:, cs],
                            start=(k == 0), stop=(k == KT - 1))
                    if dst == "q01":
                        nc.vector.tensor_copy(out=q01_sb[:, cs], in_=ps[:])
                    elif dst == "k01":
                        nc.vector.tensor_copy(out=k01_sb[:, cs], in_=ps[:])
                    else:
                        nc.vector.tensor_copy(out=q2_sb[:, cs],
                                              in_=ps[0:64, :])
                        nc.vector.tensor_copy(out=k2_sb[:, cs],
                                              in_=ps[64:128, :])
                for j in range(4 * c, 4 * c + 4):
                    pv = psum.tile([128, HPC, HD], F32, tag="op", bufs=2,
                                   name="psv")
                    for k in range(KT):
                        nc.tensor.matmul(
                            out=pv[:],
                            lhsT=xT_sb[k][:, 128 * j:128 * (j + 1)],
                            rhs=wv_sb[k][:],
                            start=(k == 0), stop=(k == KT - 1))
                    nc.vector.tensor_copy(out=v4[:, j, :, 0:HD], in_=pv[:])

            emit_proj(0)
            emit_proj(1)

            for c in range(CH):
                cs = slice(512 * c, 512 * (c + 1))
                nj = 4 * c + 4      # causal: key blocks 0 .. nj-1

                # ---- attention: scores -> exp -> (mask) -> AV, lag-1 ----
                for j in range(nj):
                    jr = j - 4 * c
                    lo = 128 * jr if jr >= 0 else 0   # shrunk col offset
                    qs = slice(512 * c + lo, 512 * (c + 1))

                    sg = psum.tile([128, HPC, 512], F32, tag="sg", bufs=1,
                                   name="sg")
                    nc.tensor.matmul(
                        out=sg[:, 0, lo:512],
                        lhsT=k01_sb[0:64, 128 * j:128 * (j + 1)],
                        rhs=q01_sb[0:64, qs], start=True, stop=True)
                    nc.tensor.matmul(
                        out=sg[:, 1, lo:512],
                        lhsT=k01_sb[64:128, 128 * j:128 * (j + 1)],
                        rhs=q01_sb[64:128, qs], start=True, stop=True)
                    nc.tensor.matmul(
                        out=sg[:, 2, lo:512],
                        lhsT=k2_sb[:, 128 * j:128 * (j + 1)],
                        rhs=q2_sb[:, qs], start=True, stop=True)

                    nc.scalar.activation(
                        out=pt4[:, j, :, lo:512], in_=sg[:, :, lo:512],
                        func=mybir.ActivationFunctionType.Exp,
                        bias=pb_sb[:, j:j + 1])
                    if jr >= 0:   # diagonal block: zero the upper triangle
                        for h in range(HPC):
                            nc.vector.tensor_mul(
                                out=pt4[:, j, h, lo:lo + 128],
                                in0=pt4[:, j, h, lo:lo + 128],
                                in1=mask_sb[:])

                # AV accumulation (after scores in priority order: the
                # scheduler interleaves it into the exp-wait gaps)
                po = [psum.tile([65, 512], F32, tag=f"po{h}", bufs=1,
                                name=f"po{h}") for h in range(HPC)]
                for j in range(nj):
                    jr = j - 4 * c
                    lo = 128 * jr if jr >= 0 else 0
                    for h in range(HPC):
                        nc.tensor.matmul(
                            out=po[h][:, lo:512],
                            lhsT=v4[:, j, h, :],
                            rhs=pt4[:, j, h, lo:512],
                            start=(j == 0), stop=(j == nj - 1))

                # projections for chunk c+2 land here in priority order:
                # PE filler while ScalarE finishes this chunk's exp stream
                if c + 2 < CH:
                    emit_proj(c + 2)

                # ---- normalize: onorm = po * recip(denominator row) ----
                # denominator rows parked at partitions 0/32/64 (32-aligned)
                for h in range(HPC):
                    nc.vector.tensor_copy(out=dn_sb[32 * h:32 * h + 1, cs],
                                          in_=po[h][64:65, :])
                nc.vector.reciprocal_approx_fast(out=rc_sb[0:65, cs],
                                                 in_=dn_sb[0:65, cs])
                # stage the 3 recip rows side by side on partition 0, then
                # one gpsimd broadcast serves all 3 heads
                rst = work.tile([1, HPC * 512], F32, tag="rst", name="rst")
                for h in range(HPC):
                    nc.vector.tensor_copy(out=rst[:, 512 * h:512 * (h + 1)],
                                          in_=rc_sb[32 * h:32 * h + 1, cs])
                bc = work.tile([65, HPC * 512], F32, tag="bc", name="bc")
                nc.gpsimd.partition_broadcast(bc[:], rst[:])
                on_dst = (onorm01[0:64, cs], onorm01[64:128, cs],
                          onorm2[0:64, cs])
                for h in range(HPC):
                    nc.vector.tensor_mul(
                        out=on_dst[h], in0=po[h][0:64, :],
                        in1=bc[0:64, 512 * h:512 * (h + 1)])

                # ---- out-projection for the 4 sq-tiles of this chunk ----
                for t in range(4 * c, 4 * c + 4):
                    ts_ = slice(128 * t, 128 * (t + 1))
                    ot = work.tile([128, D], F32, tag="ot", name="ot")
                    for n0, n1 in ((0, 512), (512, 768)):
                        pp = psum.tile([128, 512], F32, tag="op", bufs=2,
                                       name="pp")
                        nc.tensor.matmul(
                            out=pp[:, 0:n1 - n0], lhsT=onorm01[:, ts_],
                            rhs=wo01_sb[:, n0:n1], start=True, stop=False)
                        nc.tensor.matmul(
                            out=pp[:, 0:n1 - n0], lhsT=onorm2[:, ts_],
                            rhs=wo2_sb[:, n0:n1], start=False, stop=True)
                        nc.vector.tensor_copy(out=ot[:, n0:n1],
                                              in_=pp[:, 0:n1 - n0])
                    nc.sync.dma_start(out=y[ts_, :], in_=ot[:])

    nc.compile()
    return nc


def make_inputs(x, attention_mask, Wq, Wk, Wv, Wo, bo):
    """Per-core input maps (host-side sharding)."""
    bf = ml_dtypes.bfloat16
    # causal triangle 0/1 mask (shared): pass iff query-col >= key-row
    kk = np.arange(128)[:, None]
    uu = np.arange(128)[None, :]
    maskt = (uu >= kk).astype(np.float32).astype(bf)

    in_maps = []
    for core in range(NCORES):
        b, g = core // 4, core % 4
        h0, h1, h2 = range(HPC * g, HPC * (g + 1))
        xTb = np.ascontiguousarray(x[b].T).astype(bf)
        wqk = np.empty((D, HPC * 128), np.float32)
        wqk[:, 0:64] = Wq[HD * h0:HD * (h0 + 1), :].T
        wqk[:, 64:128] = Wq[HD * h1:HD * (h1 + 1), :].T
        wqk[:, 128:192] = Wk[HD * h0:HD * (h0 + 1), :].T
        wqk[:, 192:256] = Wk[HD * h1:HD * (h1 + 1), :].T
        wqk[:, 256:320] = Wq[HD * h2:HD * (h2 + 1), :].T
        wqk[:, 320:384] = Wk[HD * h2:HD * (h2 + 1), :].T
        wv_ = Wv[HD * h0:HD * (h2 + 1), :].T.copy()        # [768, 192]
        wo01_ = Wo[:, HD * h0:HD * h0 + 128].T.copy()      # [128, 768]
        wo2_ = np.zeros((65, D), np.float32)
        wo2_[0:64, :] = Wo[:, HD * h2:HD * (h2 + 1)].T
        if g == 0:  # bo must enter the partial-sum exactly once per batch
            wo2_[64, :] = bo
        # padding bias per key position (additive, pre-exp)
        pb = ((1.0 - attention_mask[b].astype(np.float32)) * NEG)
        pbias = np.ascontiguousarray(pb.reshape(SQT, 128).T)
        in_maps.append({"xT": xTb, "wqk": wqk.astype(bf),
                        "wv": wv_.astype(bf), "wo01": wo01_.astype(bf),
                        "wo2": wo2_.astype(bf), "maskt": maskt,
                        "pbias": pbias})
    return in_maps


_NC_CACHE = {}


def _get_nc():
    if "nc" not in _NC_CACHE:
        _NC_CACHE["nc"] = build_nc()
    return _NC_CACHE["nc"]


def kernel(x, attention_mask, Wq, Wk, Wv, Wo, bo, _trace=False, _trace_kwargs=None):
    x = np.asarray(x, np.float32)
    attention_mask = np.asarray(attention_mask, np.float32)
    Wq, Wk, Wv, Wo, bo = (np.asarray(a, np.float32) for a in (Wq, Wk, Wv, Wo, bo))
    nc = _get_nc()
    in_maps = make_inputs(x, attention_mask, Wq, Wk, Wv, Wo, bo)
    res = run_bass_kernel_spmd(nc, in_maps, list(range(NCORES)),
                               trace=_trace, **(_trace_kwargs or {}))
    parts = [np.asarray(res.results[i]["y"]) for i in range(NCORES)]
    out = np.stack([sum(parts[0:4]), sum(parts[4:8])]).astype(np.float32)
    if _trace:
        return out, res
    return out
